# revision 1
# baseline (speedup 1.0000x reference)
"""Trainium2 Bass kernel for nn_ConcatAttention (additive/Bahdanau attention).

Math (see reference):
  scores[t,s,b] = Va . tanh(Wt@h_t[t,b] + Ws@src[s,b] + Wa_b)
  out = softmax(scores over s)            shape (T, S, B, 1)

Sharding: data-parallel over batch B=16 -> 2 batches per core on 8 cores.
Weights replicated. All tensors fp32.

Per-core device pipeline (h/o denote the 1024-dim input/output of Wa):
  - host pre-transposes weights/inputs so every DMA load is contiguous and
    the contraction dim h lands on SBUF partitions.
  - PE: ht_proj[o,t,b], src_proj[o,s,b] (matmuls, o on partitions)
  - DVE/GPSIMD: X[o,(t,s)] = ht_proj[o,t]+Wa_b[o] + src_proj[o,s] via
    broadcast (stride-0) tensor_tensor adds
  - ACT: tanh(X)  (the dominant cost: 8.4M elems/core)
  - PE: scores = Va^T @ tanh  (M=32 zero-padded Va; 16 accumulation groups
    packed 4 row-groups x 4 cols into one (128,2048) PSUM region = 4 banks)
  - ACT exp -> DVE row sums/reciprocal/scale -> DMA out (T,BS,S) staging
  - host: transpose/concat core outputs -> (T,S,B,1)
"""

import numpy as np

T, S, B, H = 32, 128, 16, 1024
NCORES = 8
BS = B // NCORES          # batches per core
P = 128                   # partitions
HC = H // P               # h chunks
OC = H // P               # o chunks
TS = T * S                # 4096 free elements per (b, oc) tile

# (b, oc) X-build units executed on GPSIMD instead of DVE (load balance:
# DVE ~4.4us/unit, GPSIMD ~8.9us/unit, DVE also does evacs + softmax).
GPSIMD_OCS = (1, 3, 5)

_CACHE = {}


def _build_nc():
    import concourse.bacc as bacc
    import concourse.mybir as mybir
    import concourse.tile as tile
    from concourse._compat import axon_active

    f32 = mybir.dt.float32
    AF = mybir.ActivationFunctionType
    ALU = mybir.AluOpType

    nc = bacc.Bacc(
        "TRN2",
        target_bir_lowering=False,
        debug=False,
        enable_partition_id=False,
    )

    # DRAM I/O (host-side prepped layouts; h contiguous -> partition dim)
    d_wtT = nc.dram_tensor("wtT", (H, H), f32, kind="ExternalInput")      # [h, o]
    d_wsT = nc.dram_tensor("wsT", (H, H), f32, kind="ExternalInput")      # [h, o]
    d_htT = nc.dram_tensor("htT", (H, BS, T), f32, kind="ExternalInput")  # [h, b, t]
    d_srcT = nc.dram_tensor("srcT", (H, BS, S), f32, kind="ExternalInput")  # [h,b,s]
    d_wab = nc.dram_tensor("wab", (H,), f32, kind="ExternalInput")
    d_va = nc.dram_tensor("va", (H,), f32, kind="ExternalInput")
    d_out = nc.dram_tensor("out", (T, BS, S), f32, kind="ExternalOutput")

    with tile.TileContext(nc) as tc:
        with (
            tc.tile_pool(name="consts", bufs=1) as consts,
            tc.tile_pool(name="wpool", bufs=2) as wpool,
            tc.tile_pool(name="proj", bufs=1) as proj,
            tc.tile_pool(name="xpool", bufs=2) as xpool,
            tc.tile_pool(name="hpool", bufs=3) as hpool,
            tc.tile_pool(name="spool", bufs=1) as spool,
            tc.tile_pool(name="ps_ht", bufs=2, space="PSUM") as ps_ht,
            tc.tile_pool(name="ps_src", bufs=2, space="PSUM") as ps_src,
            tc.tile_pool(name="ps_sc", bufs=1, space="PSUM") as ps_sc,
        ):
            # ---- constant / input loads (HWDGE) ----
            sb_htT = consts.tile([P, HC, BS, T], f32)
            nc.sync.dma_start(
                out=sb_htT, in_=d_htT.ap().rearrange("(hc p) b t -> p hc b t", p=P)
            )
            sb_wab = consts.tile([P, OC], f32)
            nc.sync.dma_start(
                out=sb_wab, in_=d_wab.ap().rearrange("(oc p) -> p oc", p=P)
            )
            sb_va = consts.tile([P, OC, 1], f32)
            nc.sync.dma_start(
                out=sb_va,
                in_=d_va.ap().rearrange("(oc p) -> p oc", p=P).unsqueeze(2),
            )
            sb_zero = consts.tile([P, P], f32)  # zero lhsT for psum-bank init
            nc.vector.memset(sb_zero, 0.0)
            sb_srcT = consts.tile([P, HC, BS, S], f32)
            nc.sync.dma_start(
                out=sb_srcT, in_=d_srcT.ap().rearrange("(hc p) b s -> p hc b s", p=P)
            )

            wtT_v = d_wtT.ap().rearrange("(hc p) o -> p hc o", p=P)
            wsT_v = d_wsT.ap().rearrange("(hc p) o -> p hc o", p=P)

            # ---- phase 1: projections (o on partitions) ----
            ht_projb = proj.tile([P, OC, BS, T], f32)   # ht_proj + Wa_b
            src_sb = proj.tile([P, OC, BS, S], f32)     # src_proj
            for oc in range(OC):
                wt = wpool.tile([P, HC, P], f32, tag="wt")
                nc.sync.dma_start(out=wt, in_=wtT_v[:, :, oc * P:(oc + 1) * P])
                ws = wpool.tile([P, HC, P], f32, tag="ws")
                nc.sync.dma_start(out=ws, in_=wsT_v[:, :, oc * P:(oc + 1) * P])

                htp = ps_ht.tile([P, BS * T], f32, tag="htp")
                for hc in range(HC):
                    nc.tensor.matmul(
                        htp,
                        lhsT=wt[:, hc, :],
                        rhs=sb_htT[:, hc, :, :],
                        start=(hc == 0),
                        stop=(hc == HC - 1),
                    )
                # evacuate + fold bias (per-partition scalar add)
                nc.vector.tensor_scalar(
                    out=ht_projb[:, oc, :, :],
                    in0=htp.rearrange("p (b t) -> p b t", b=BS),
                    scalar1=sb_wab[:, oc:oc + 1],
                    scalar2=None,
                    op0=ALU.add,
                )

                srp = ps_src.tile([P, BS * S], f32, tag="srp")
                for hc in range(HC):
                    nc.tensor.matmul(
                        srp,
                        lhsT=ws[:, hc, :],
                        rhs=sb_srcT[:, hc, :, :],
                        start=(hc == 0),
                        stop=(hc == HC - 1),
                    )
                nc.vector.tensor_copy(
                    src_sb[:, oc, :, :], srp.rearrange("p (b s) -> p b s", b=BS)
                )

            # ---- phases 2+3: X build -> tanh -> score matmuls ----
            # scores psum: one (128, 1024) tile (2 banks) per b. Block
            # (b, k): row 32*(k%4), cols 512*(k//4)..+512. Each bank's
            # accumulation group is opened ONCE by a dummy all-zero M=128
            # matmul (start=True, writes every row -> has_written set
            # everywhere); the real M=1 Va matmuls then accumulate with
            # start=False. Correct under both whole-bank and per-partition
            # has_written-clear semantics, and keeps one group per bank.
            sc_ps = [
                ps_sc.tile([P, 1024], f32, tag=f"scb{b}", name=f"scb{b}")
                for b in range(BS)
            ]

            for b in range(BS):
                for h4 in range(2):  # open each bank's group with zeros
                    nc.tensor.matmul(
                        sc_ps[b][:, 512 * h4:512 * (h4 + 1)],
                        lhsT=sb_zero,
                        rhs=sb_srcT[:, 0:2, :, :],
                        start=True,
                        stop=False,
                        skip_group_check=True,
                    )
                for oc in range(OC):
                    ht_b = ht_projb[:, oc, b, :].unsqueeze(2).broadcast_to((P, T, S))
                    src_b = src_sb[:, oc, b, :].unsqueeze(1).broadcast_to((P, T, S))
                    x = xpool.tile([P, T, S], f32,
                                   tag="xg" if oc in GPSIMD_OCS else "xd")
                    if oc in GPSIMD_OCS:
                        nc.gpsimd.tensor_tensor(out=x, in0=ht_b, in1=src_b, op=ALU.add)
                    else:
                        nc.vector.tensor_tensor(out=x, in0=ht_b, in1=src_b, op=ALU.add)

                    h_tile = hpool.tile([P, TS], f32, tag="h")
                    nc.scalar.activation(
                        out=h_tile, in_=x.rearrange("p t s -> p (t s)"), func=AF.Tanh
                    )

                    for k in range(8):
                        j = k % 4
                        h4 = k // 4
                        nc.tensor.matmul(
                            sc_ps[b][32 * j:32 * j + 1,
                                     512 * h4:512 * (h4 + 1)],
                            lhsT=sb_va[:, oc, :],
                            rhs=h_tile[:, 512 * k:512 * (k + 1)],
                            start=False,
                            stop=(oc == OC - 1 and j == 3),
                            tile_position=(0, 32 * j),
                            skip_group_check=True,
                        )

                # ---- softmax over s for this b (cols 1024b..1024b+1024) ----
                ee = spool.tile([P, 8, S], f32, tag=f"ee{b}")
                nc.scalar.activation(
                    out=ee.rearrange("p g s -> p (g s)"),
                    in_=sc_ps[b],
                    func=AF.Exp,
                )
                sums = spool.tile([P, 8], f32, tag=f"sums{b}")
                nc.vector.reduce_sum(sums.unsqueeze(2), ee, axis=mybir.AxisListType.X)
                rec = spool.tile([P, 8], f32, tag=f"rec{b}")
                nc.vector.reciprocal(out=rec, in_=sums)
                en = spool.tile([P, 8, S], f32, tag=f"en{b}")
                nc.vector.tensor_tensor(
                    out=en,
                    in0=ee,
                    in1=rec.unsqueeze(2).broadcast_to((P, 8, S)),
                    op=ALU.mult,
                )
                # out[t, b, s] with t = 16*k4 + 4*j + r2; en rows 32j hold
                # (k4, r2, s) at free (k4*4 + r2, s). DMA APs max 3 dims ->
                # one DMA per k4 half.
                for k4 in range(2):
                    src_view = en[0:P:32, 4 * k4:4 * (k4 + 1), :]
                    dst_view = d_out.ap().rearrange(
                        "(k4 j r2) bb s -> k4 j r2 bb s", k4=2, j=4
                    )[k4, :, :, b, :]
                    nc.sync.dma_start(out=dst_view, in_=src_view)

    nc.compile()
    return nc


def _prep_in_maps(h_t, src_encodings, Wa_w, Wa_b, Va_w):
    h_t = np.asarray(h_t, dtype=np.float32)
    src_encodings = np.asarray(src_encodings, dtype=np.float32)
    Wa_w = np.asarray(Wa_w, dtype=np.float32)
    Wa_b = np.asarray(Wa_b, dtype=np.float32)
    Va_w = np.asarray(Va_w, dtype=np.float32)

    wtT = np.ascontiguousarray(Wa_w[:, :H].T)   # [h, o]
    wsT = np.ascontiguousarray(Wa_w[:, H:].T)   # [h, o]
    va = np.ascontiguousarray(Va_w[0])
    in_maps = []
    for c in range(NCORES):
        sl = slice(c * BS, (c + 1) * BS)
        htT = np.ascontiguousarray(h_t[:, sl, :].transpose(2, 1, 0))          # h,b,t
        srcT = np.ascontiguousarray(src_encodings[:, sl, :].transpose(2, 1, 0))
        in_maps.append({
            "wtT": wtT, "wsT": wsT, "htT": htT, "srcT": srcT,
            "wab": Wa_b, "va": va,
        })
    return in_maps


def _gather(results):
    # per-core out: (T, BS, S) -> full (T, S, B, 1)
    outs = [r["out"] for r in results]
    full = np.concatenate([o.transpose(0, 2, 1) for o in outs], axis=2)
    return np.ascontiguousarray(full[..., None])


def kernel(h_t, src_encodings, Wa_w, Wa_b, Va_w):
    from concourse import bass_utils

    if "nc" not in _CACHE:
        _CACHE["nc"] = _build_nc()
    nc = _CACHE["nc"]
    in_maps = _prep_in_maps(h_t, src_encodings, Wa_w, Wa_b, Va_w)
    res = bass_utils.run_bass_kernel_spmd(nc, in_maps, core_ids=list(range(NCORES)))
    return _gather(res.results)


if __name__ == "__main__":
    # CoreSim check of core 0 against numpy
    from concourse.bass_interp import CoreSim

    rng = np.random.default_rng(0)
    w_scale = 1.0 / np.sqrt(2 * H)
    h_t = rng.standard_normal((T, B, H), dtype=np.float32)
    src = rng.standard_normal((S, B, H), dtype=np.float32)
    Wa_w = rng.standard_normal((H, 2 * H), dtype=np.float32) * w_scale
    Wa_b = rng.standard_normal((H,), dtype=np.float32) * w_scale
    Va_w = rng.standard_normal((1, H), dtype=np.float32) / np.sqrt(H)

    nc = _build_nc()
    in_maps = _prep_in_maps(h_t, src, Wa_w, Wa_b, Va_w)
    sim = CoreSim(nc)
    for k, v in in_maps[0].items():
        sim.tensor(k)[:] = v
    sim.simulate(check_with_hw=False)
    got = sim.tensor("out")  # (T, BS, S)

    # numpy reference for core 0
    Wt, Ws = Wa_w[:, :H], Wa_w[:, H:]
    hp = np.einsum("tbh,oh->tbo", h_t[:, :BS], Wt)
    sp = np.einsum("sbh,oh->sbo", src[:, :BS], Ws)
    hid = np.tanh(hp[:, None] + sp[None] + Wa_b)
    sc = np.einsum("tsbh,oh->tsbo", hid, Va_w)[..., 0]  # (T,S,BS)
    e = np.exp(sc - sc.max(axis=1, keepdims=True))
    ref = e / e.sum(axis=1, keepdims=True)              # (T,S,BS)
    ref_stage = ref.transpose(0, 2, 1)                  # (T,BS,S)

    err = np.abs(got - ref_stage)
    rel = err.max() / np.abs(ref_stage).max()
    print("max abs err:", err.max(), " rel:", rel)
    assert rel < 2e-5, "mismatch"
    print("SIM OK")



# revision 3
# speedup vs baseline: 4.1626x; 4.1626x over previous
"""Trainium2 Bass kernel for nn_ConcatAttention (additive/Bahdanau attention).

Math (see reference):
  scores[t,s,b] = Va . tanh(Wt@h_t[t,b] + Ws@src[s,b] + Wa_b)
  out = softmax(scores over s)            shape (T, S, B, 1)

Sharding: data-parallel over batch B=16 -> 2 batches per core on 8 cores.

Wire-format optimizations (the end-to-end time here is dominated by the
host->device transport, not device compute):
  - h_t / src_encodings / weights ship as bf16 (PSUM accumulation stays
    fp32; empirical rel err ~2e-3 vs the 2e-2 gate).
  - the 2M-param Wa weight is NOT replicated to all 8 cores: each core
    receives a distinct 1/8 row-shard of W2 = vstack(WtT, WsT) (512 KB
    bf16) and the full 4 MB weight is reassembled on-device with a DRAM
    AllGather over NeuronLink before the projection matmuls.
  - the dispatcher jit(shard_map(...)) is built once and cached, so
    repeat calls skip retrace/relower (~200 ms/call saved).
  Net: ~78 MB shipped per call (fp32, replicated weights) -> ~9.8 MB.

Per-core device pipeline (h/o denote the 1024-dim input/output of Wa):
  - PE: ht_proj[o,t,b], src_proj[o,s,b] (bf16 matmuls, o on partitions,
    fp32 PSUM)
  - DVE/GPSIMD: X[o,(t,s)] = ht_proj[o,t]+Wa_b[o] + src_proj[o,s] via
    broadcast (stride-0) tensor_tensor adds (fp32)
  - ACT: tanh(X)  (the dominant device cost: 8.4M elems/core)
  - PE: scores = Va^T @ tanh  (M=1 Va rows; 16 accumulation groups packed
    4 row-groups x 4 cols into one (128,1024) PSUM region per batch)
  - ACT exp -> DVE row sums/reciprocal/scale -> DMA out (T,BS,S) staging
  - host: transpose/concat core outputs -> (T,S,B,1)
"""

import numpy as np
import ml_dtypes

T, S, B, H = 32, 128, 16, 1024
NCORES = 8
BS = B // NCORES          # batches per core
P = 128                   # partitions
HC = H // P               # h chunks
OC = H // P               # o chunks
TS = T * S                # 4096 free elements per (b, oc) tile
WSH = 2 * H // NCORES     # weight shard rows per core (of vstack(WtT, WsT))

# (b, oc) X-build units executed on GPSIMD instead of DVE (load balance:
# DVE ~4.4us/unit, GPSIMD ~8.9us/unit, DVE also does evacs + softmax).
GPSIMD_OCS = (1, 3, 5)

_CACHE = {}


def _build_nc(gather=True):
    import concourse.bacc as bacc
    import concourse.mybir as mybir
    import concourse.tile as tile

    f32 = mybir.dt.float32
    bf16 = mybir.dt.bfloat16
    AF = mybir.ActivationFunctionType
    ALU = mybir.AluOpType

    nc = bacc.Bacc(
        "TRN2",
        target_bir_lowering=False,
        debug=False,
        enable_partition_id=False,
        num_devices=NCORES if gather else None,
    )

    # DRAM I/O (host-side prepped layouts; h contiguous -> partition dim)
    if gather:
        # per-core row-shard of W2 = vstack(WtT, WsT), bf16
        d_wsh = nc.dram_tensor("w2sh", (WSH, H), bf16, kind="ExternalInput")
    else:
        # CoreSim variant: full weight as a direct input (no collective)
        d_w2 = nc.dram_tensor("w2", (2 * H, H), bf16, kind="ExternalInput")
    d_htT = nc.dram_tensor("htT", (H, BS, T), bf16, kind="ExternalInput")  # [h,b,t]
    d_srcT = nc.dram_tensor("srcT", (H, BS, S), bf16, kind="ExternalInput")
    d_wab = nc.dram_tensor("wab", (H,), f32, kind="ExternalInput")
    d_va = nc.dram_tensor("va", (H,), f32, kind="ExternalInput")
    d_out = nc.dram_tensor("out", (T, BS, S), f32, kind="ExternalOutput")

    with tile.TileContext(nc) as tc:
        with (
            tc.tile_pool(name="dram", bufs=1, space="DRAM") as dram,
            tc.tile_pool(name="consts", bufs=1) as consts,
            tc.tile_pool(name="wpool", bufs=2) as wpool,
            tc.tile_pool(name="proj", bufs=1) as proj,
            tc.tile_pool(name="xpool", bufs=2) as xpool,
            tc.tile_pool(name="hpool", bufs=3) as hpool,
            tc.tile_pool(name="spool", bufs=1) as spool,
            tc.tile_pool(name="ps_ht", bufs=2, space="PSUM") as ps_ht,
            tc.tile_pool(name="ps_src", bufs=2, space="PSUM") as ps_src,
            tc.tile_pool(name="ps_sc", bufs=1, space="PSUM") as ps_sc,
        ):
            # ---- phase 0: reassemble full weight on-device ----
            if gather:
                b_in = dram.tile([WSH, H], bf16)
                w2 = dram.tile([2 * H, H], bf16)
                nc.sync.dma_start(out=b_in, in_=d_wsh.ap())
                nc.gpsimd.collective_compute(
                    "AllGather",
                    mybir.AluOpType.bypass,
                    replica_groups=[list(range(NCORES))],
                    ins=[b_in[:].opt()],
                    outs=[w2[:].opt()],
                )
                w2v = w2[:]
            else:
                w2v = d_w2.ap()
            # W2 row h<H is WtT[h, :]; row H+h is WsT[h, :]
            w2r = w2v.rearrange("(g hc p) o -> g p hc o", g=2, p=P)

            # ---- constant / input loads (HWDGE) ----
            sb_htT = consts.tile([P, HC, BS, T], bf16)
            nc.sync.dma_start(
                out=sb_htT, in_=d_htT.ap().rearrange("(hc p) b t -> p hc b t", p=P)
            )
            sb_wab = consts.tile([P, OC], f32)
            nc.sync.dma_start(
                out=sb_wab, in_=d_wab.ap().rearrange("(oc p) -> p oc", p=P)
            )
            sb_va = consts.tile([P, OC, 1], f32)
            nc.sync.dma_start(
                out=sb_va,
                in_=d_va.ap().rearrange("(oc p) -> p oc", p=P).unsqueeze(2),
            )
            sb_zero = consts.tile([P, P], bf16)  # zero lhsT for psum-bank init
            nc.vector.memset(sb_zero, 0.0)
            sb_srcT = consts.tile([P, HC, BS, S], bf16)
            nc.sync.dma_start(
                out=sb_srcT, in_=d_srcT.ap().rearrange("(hc p) b s -> p hc b s", p=P)
            )

            # ---- phase 1: projections (o on partitions) ----
            ht_projb = proj.tile([P, OC, BS, T], f32)   # ht_proj + Wa_b
            src_sb = proj.tile([P, OC, BS, S], f32)     # src_proj
            for oc in range(OC):
                wt = wpool.tile([P, HC, P], bf16, tag="wt")
                nc.sync.dma_start(out=wt, in_=w2r[0, :, :, oc * P:(oc + 1) * P])
                ws = wpool.tile([P, HC, P], bf16, tag="ws")
                nc.sync.dma_start(out=ws, in_=w2r[1, :, :, oc * P:(oc + 1) * P])

                htp = ps_ht.tile([P, BS * T], f32, tag="htp")
                for hc in range(HC):
                    nc.tensor.matmul(
                        htp,
                        lhsT=wt[:, hc, :],
                        rhs=sb_htT[:, hc, :, :],
                        start=(hc == 0),
                        stop=(hc == HC - 1),
                    )
                # evacuate + fold bias (per-partition scalar add)
                nc.vector.tensor_scalar(
                    out=ht_projb[:, oc, :, :],
                    in0=htp.rearrange("p (b t) -> p b t", b=BS),
                    scalar1=sb_wab[:, oc:oc + 1],
                    scalar2=None,
                    op0=ALU.add,
                )

                srp = ps_src.tile([P, BS * S], f32, tag="srp")
                for hc in range(HC):
                    nc.tensor.matmul(
                        srp,
                        lhsT=ws[:, hc, :],
                        rhs=sb_srcT[:, hc, :, :],
                        start=(hc == 0),
                        stop=(hc == HC - 1),
                    )
                nc.vector.tensor_copy(
                    src_sb[:, oc, :, :], srp.rearrange("p (b s) -> p b s", b=BS)
                )

            # ---- phases 2+3: X build -> tanh -> score matmuls ----
            # scores psum: one (128, 1024) tile (2 banks) per b. Block
            # (b, k): row 32*(k%4), cols 512*(k//4)..+512. Each bank's
            # accumulation group is opened ONCE by a dummy all-zero M=128
            # matmul (start=True, writes every row -> has_written set
            # everywhere); the real M=1 Va matmuls then accumulate with
            # start=False. Correct under both whole-bank and per-partition
            # has_written-clear semantics, and keeps one group per bank.
            sc_ps = [
                ps_sc.tile([P, 1024], f32, tag=f"scb{b}", name=f"scb{b}")
                for b in range(BS)
            ]

            for b in range(BS):
                for h4 in range(2):  # open each bank's group with zeros
                    nc.tensor.matmul(
                        sc_ps[b][:, 512 * h4:512 * (h4 + 1)],
                        lhsT=sb_zero,
                        rhs=sb_srcT[:, 0:2, :, :],
                        start=True,
                        stop=False,
                        skip_group_check=True,
                    )
                for oc in range(OC):
                    ht_b = ht_projb[:, oc, b, :].unsqueeze(2).broadcast_to((P, T, S))
                    src_b = src_sb[:, oc, b, :].unsqueeze(1).broadcast_to((P, T, S))
                    x = xpool.tile([P, T, S], f32,
                                   tag="xg" if oc in GPSIMD_OCS else "xd")
                    if oc in GPSIMD_OCS:
                        nc.gpsimd.tensor_tensor(out=x, in0=ht_b, in1=src_b, op=ALU.add)
                    else:
                        nc.vector.tensor_tensor(out=x, in0=ht_b, in1=src_b, op=ALU.add)

                    h_tile = hpool.tile([P, TS], f32, tag="h")
                    nc.scalar.activation(
                        out=h_tile, in_=x.rearrange("p t s -> p (t s)"), func=AF.Tanh
                    )

                    for k in range(8):
                        j = k % 4
                        h4 = k // 4
                        nc.tensor.matmul(
                            sc_ps[b][32 * j:32 * j + 1,
                                     512 * h4:512 * (h4 + 1)],
                            lhsT=sb_va[:, oc, :],
                            rhs=h_tile[:, 512 * k:512 * (k + 1)],
                            start=False,
                            stop=(oc == OC - 1 and j == 3),
                            tile_position=(0, 32 * j),
                            skip_group_check=True,
                        )

                # ---- softmax over s for this b (cols 1024b..1024b+1024) ----
                ee = spool.tile([P, 8, S], f32, tag=f"ee{b}")
                nc.scalar.activation(
                    out=ee.rearrange("p g s -> p (g s)"),
                    in_=sc_ps[b],
                    func=AF.Exp,
                )
                sums = spool.tile([P, 8], f32, tag=f"sums{b}")
                nc.vector.reduce_sum(sums.unsqueeze(2), ee, axis=mybir.AxisListType.X)
                rec = spool.tile([P, 8], f32, tag=f"rec{b}")
                nc.vector.reciprocal(out=rec, in_=sums)
                en = spool.tile([P, 8, S], f32, tag=f"en{b}")
                nc.vector.tensor_tensor(
                    out=en,
                    in0=ee,
                    in1=rec.unsqueeze(2).broadcast_to((P, 8, S)),
                    op=ALU.mult,
                )
                # out[t, b, s] with t = 16*k4 + 4*j + r2; en rows 32j hold
                # (k4, r2, s) at free (k4*4 + r2, s). DMA APs max 3 dims ->
                # one DMA per k4 half.
                for k4 in range(2):
                    src_view = en[0:P:32, 4 * k4:4 * (k4 + 1), :]
                    dst_view = d_out.ap().rearrange(
                        "(k4 j r2) bb s -> k4 j r2 bb s", k4=2, j=4
                    )[k4, :, :, b, :]
                    nc.sync.dma_start(out=dst_view, in_=src_view)

    nc.compile()
    return nc


def _prep_in_maps(h_t, src_encodings, Wa_w, Wa_b, Va_w):
    bf16 = ml_dtypes.bfloat16
    h_t = np.asarray(h_t, dtype=np.float32)
    src_encodings = np.asarray(src_encodings, dtype=np.float32)
    Wa_w = np.asarray(Wa_w, dtype=np.float32)
    Wa_b = np.asarray(Wa_b, dtype=np.float32)
    Va_w = np.asarray(Va_w, dtype=np.float32)

    # W2 = vstack(WtT, WsT) = Wa_w.T, bf16; core c ships rows [WSH*c : WSH*(c+1)]
    w2 = Wa_w.T.astype(bf16, order="C")
    va = np.ascontiguousarray(Va_w[0])
    htT_full = h_t.transpose(2, 1, 0).astype(bf16, order="C")       # (H, B, T)
    srcT_full = src_encodings.transpose(2, 1, 0).astype(bf16, order="C")  # (H, B, S)
    in_maps = []
    for c in range(NCORES):
        sl = slice(c * BS, (c + 1) * BS)
        in_maps.append({
            "w2sh": w2[WSH * c:WSH * (c + 1)],
            "htT": htT_full[:, sl, :],
            "srcT": srcT_full[:, sl, :],
            "wab": Wa_b, "va": va,
        })
    return in_maps


def _make_dispatch(nc):
    """Build a cached jit(shard_map) dispatcher mirroring
    bass_utils.run_bass_kernel_spmd's axon path (bass2jax.run_bass_via_pjrt),
    but constructed once so repeat calls hit the jit cache instead of
    retracing + relowering (~200 ms/call)."""
    import jax
    import numpy as _np
    from jax.sharding import Mesh, PartitionSpec
    try:
        from jax import shard_map as _shard_map_mod  # jax >= 0.8
        shard_map = _shard_map_mod
    except ImportError:
        from jax.experimental.shard_map import shard_map
    from concourse import bass2jax, mybir

    bass2jax.install_neuronx_cc_hook()

    in_names, out_names, out_avals, zero_shapes = [], [], [], []
    for alloc in nc.m.functions[0].allocations:
        if not isinstance(alloc, mybir.MemoryLocationSet):
            continue
        name = alloc.memorylocations[0].name
        if alloc.kind == "ExternalInput":
            in_names.append(name)
        elif alloc.kind == "ExternalOutput":
            out_names.append(name)
            shape = tuple(alloc.tensor_shape)
            dtype = mybir.dt.np(alloc.dtype)
            out_avals.append(jax.core.ShapedArray(shape, dtype))
            zero_shapes.append((shape, dtype))
    n_params = len(in_names)
    all_names = tuple(in_names) + tuple(out_names)
    donate = tuple(range(n_params, n_params + len(out_names)))

    def _body(*args):
        outs = bass2jax._bass_exec_p.bind(
            *args,
            out_avals=tuple(out_avals),
            in_names=all_names,
            out_names=tuple(out_names),
            lowering_input_output_aliases=(),
            sim_require_finite=True,
            sim_require_nnan=True,
            nc=nc,
        )
        return tuple(outs)

    devices = jax.devices()[:NCORES]
    mesh = Mesh(_np.asarray(devices), ("core",))
    nin = n_params + len(out_names)
    sharded = jax.jit(
        shard_map(
            _body, mesh=mesh, in_specs=(PartitionSpec("core"),) * nin,
            out_specs=(PartitionSpec("core"),) * len(out_names), check_rep=False,
        ),
        donate_argnums=donate, keep_unused=True,
    )

    def dispatch(in_maps):
        concat_in = [
            np.concatenate([np.asarray(m[n]) for m in in_maps], axis=0)
            for n in in_names
        ]
        concat_zeros = [
            np.zeros((NCORES * s[0], *s[1:]), dt) for s, dt in zero_shapes
        ]
        out_arrs = sharded(*concat_in, *concat_zeros)
        return [
            {
                n: np.asarray(out_arrs[i]).reshape(NCORES, *out_avals[i].shape)[c]
                for i, n in enumerate(out_names)
            }
            for c in range(NCORES)
        ]

    return dispatch


def _gather(results):
    # per-core out: (T, BS, S) -> full (T, S, B, 1)
    outs = [r["out"] for r in results]
    full = np.concatenate([o.transpose(0, 2, 1) for o in outs], axis=2)
    return np.ascontiguousarray(full[..., None])


def kernel(h_t, src_encodings, Wa_w, Wa_b, Va_w):
    if "nc" not in _CACHE:
        _CACHE["nc"] = _build_nc()
    if "dispatch" not in _CACHE:
        try:
            _CACHE["dispatch"] = _make_dispatch(_CACHE["nc"])
        except Exception:
            from concourse import bass_utils

            def _fallback(in_maps):
                res = bass_utils.run_bass_kernel_spmd(
                    _CACHE["nc"], in_maps, core_ids=list(range(NCORES))
                )
                return res.results
            _CACHE["dispatch"] = _fallback
    in_maps = _prep_in_maps(h_t, src_encodings, Wa_w, Wa_b, Va_w)
    return _gather(_CACHE["dispatch"](in_maps))


if __name__ == "__main__":
    # CoreSim check of core 0 against numpy (gather=False variant: the
    # compute pipeline is identical; the AllGather is validated on HW)
    from concourse.bass_interp import CoreSim

    rng = np.random.default_rng(0)
    w_scale = 1.0 / np.sqrt(2 * H)
    h_t = rng.standard_normal((T, B, H), dtype=np.float32)
    src = rng.standard_normal((S, B, H), dtype=np.float32)
    Wa_w = rng.standard_normal((H, 2 * H), dtype=np.float32) * w_scale
    Wa_b = rng.standard_normal((H,), dtype=np.float32) * w_scale
    Va_w = rng.standard_normal((1, H), dtype=np.float32) / np.sqrt(H)

    nc = _build_nc(gather=False)
    in_maps = _prep_in_maps(h_t, src, Wa_w, Wa_b, Va_w)
    sim = CoreSim(nc)
    w2_full = np.concatenate([in_maps[c]["w2sh"] for c in range(NCORES)], axis=0)
    sim.tensor("w2")[:] = w2_full
    for k in ("htT", "srcT", "wab", "va"):
        sim.tensor(k)[:] = in_maps[0][k]
    sim.simulate(check_with_hw=False)
    got = sim.tensor("out")  # (T, BS, S)

    # numpy reference for core 0 (bf16 wire precision)
    f32 = np.float32
    Wt = Wa_w[:, :H].astype(ml_dtypes.bfloat16).astype(f32)
    Ws = Wa_w[:, H:].astype(ml_dtypes.bfloat16).astype(f32)
    htq = h_t[:, :BS].astype(ml_dtypes.bfloat16).astype(f32)
    srcq = src[:, :BS].astype(ml_dtypes.bfloat16).astype(f32)
    hp = np.einsum("tbh,oh->tbo", htq, Wt)
    sp = np.einsum("sbh,oh->sbo", srcq, Ws)
    hid = np.tanh(hp[:, None] + sp[None] + Wa_b)
    sc = np.einsum("tsbh,oh->tsbo", hid, Va_w)[..., 0]  # (T,S,BS)
    e = np.exp(sc - sc.max(axis=1, keepdims=True))
    ref = e / e.sum(axis=1, keepdims=True)              # (T,S,BS)
    ref_stage = ref.transpose(0, 2, 1)                  # (T,BS,S)

    err = np.abs(got - ref_stage)
    rel = err.max() / np.abs(ref_stage).max()
    print("max abs err:", err.max(), " rel:", rel)
    assert rel < 2e-4, "mismatch"
    print("SIM OK")


# revision 4
# speedup vs baseline: 5.3265x; 1.2796x over previous
"""Trainium2 Bass kernel for nn_ConcatAttention (additive/Bahdanau attention).

Math (see reference):
  scores[t,s,b] = Va . tanh(Wt@h_t[t,b] + Ws@src[s,b] + Wa_b)
  out = softmax(scores over s)            shape (T, S, B, 1)

Sharding: data-parallel over batch B=16 -> 2 batches per core on 8 cores.

Wire/format optimizations (end-to-end time is dominated by the host->device
transport, not device compute):
  - h_t / src_encodings / weights ship as bf16 in NATIVE layout (host prep
    is just contiguous casts + big-block permutes, ~5 ms); the h->partition
    transposes happen on-device via crossbar transpose-DMAs
    (dma_start_transpose, 16x128 xbar tiles).
  - the 2M-param Wa weight is NOT replicated to all 8 cores: each core
    receives a distinct 1/8 row-shard (512 KB bf16) and the full 4 MB
    weight is reassembled on-device with a DRAM AllGather over NeuronLink
    before the projection matmuls.
  - output ships back as bf16 (softmax probs; quantization ~4e-4 abs).
  - the dispatcher jit(shard_map(...)) is built once and cached, so repeat
    calls skip retrace/relower (~200 ms/call); inputs are passed as
    pre-concatenated globals so no per-call np.concatenate.
  Net: ~78 MB shipped per call (fp32, replicated weights) -> ~9.2 MB.

Per-core device pipeline (h/o denote the 1024-dim input/output of Wa):
  - PE: ht_proj[o,t,b], src_proj[o,s,b] (bf16 matmuls, o on partitions,
    fp32 PSUM)
  - DVE/GPSIMD: X[o,(t,s)] = ht_proj[o,t]+Wa_b[o] + src_proj[o,s] via
    broadcast (stride-0) tensor_tensor adds (fp32)
  - ACT: tanh(X)  (the dominant device cost: 8.4M elems/core)
  - PE: scores = Va^T @ tanh  (M=1 Va rows; 16 accumulation groups packed
    4 row-groups x 4 cols into one (128,1024) PSUM region per batch)
  - ACT exp -> DVE row sums/reciprocal/scale (bf16 out) -> DMA out
    (T,BS,S) staging; host: transpose/concat core outputs -> (T,S,B,1)
"""

import numpy as np
import ml_dtypes

T, S, B, H = 32, 128, 16, 1024
NCORES = 8
BS = B // NCORES          # batches per core
P = 128                   # partitions
HC = H // P               # h chunks
OC = H // P               # o chunks
TS = T * S                # 4096 free elements per (b, oc) tile
WSH = H // NCORES         # weight shard rows per core (o-rows of Wa_w)

# (b, oc) X-build units executed on GPSIMD instead of DVE (load balance:
# DVE ~4.4us/unit, GPSIMD ~8.9us/unit, DVE also does evacs + softmax).
GPSIMD_OCS = (1, 3, 5)

_CACHE = {}


def _build_nc(gather=True):
    import concourse.bacc as bacc
    import concourse.mybir as mybir
    import concourse.tile as tile

    f32 = mybir.dt.float32
    bf16 = mybir.dt.bfloat16
    AF = mybir.ActivationFunctionType
    ALU = mybir.AluOpType

    nc = bacc.Bacc(
        "TRN2",
        target_bir_lowering=False,
        debug=False,
        enable_partition_id=False,
        num_devices=NCORES if gather else None,
    )

    # DRAM I/O — all in NATIVE layouts (host does no transposes)
    if gather:
        # per-core o-row shard of Wa_w: rows [WSH*c, WSH*(c+1)), bf16
        d_wsh = nc.dram_tensor("w2sh", (WSH, 2 * H), bf16, kind="ExternalInput")
    else:
        # CoreSim variant: full weight as a direct input (no collective)
        d_w2 = nc.dram_tensor("w2", (H, 2 * H), bf16, kind="ExternalInput")
    d_htN = nc.dram_tensor("htN", (T, BS, H), bf16, kind="ExternalInput")
    d_srcN = nc.dram_tensor("srcN", (S, BS, H), bf16, kind="ExternalInput")
    d_wab = nc.dram_tensor("wab", (H,), f32, kind="ExternalInput")
    d_va = nc.dram_tensor("va", (H,), f32, kind="ExternalInput")
    d_out = nc.dram_tensor("out", (T, BS, S), bf16, kind="ExternalOutput")

    with tile.TileContext(nc) as tc:
        with (
            tc.tile_pool(name="dram", bufs=1, space="DRAM") as dram,
            tc.tile_pool(name="consts", bufs=1) as consts,
            tc.tile_pool(name="proj", bufs=1) as proj,
            tc.tile_pool(name="xpool", bufs=2) as xpool,
            tc.tile_pool(name="hpool", bufs=3) as hpool,
            tc.tile_pool(name="spool", bufs=1) as spool,
            tc.tile_pool(name="ps_ht", bufs=2, space="PSUM") as ps_ht,
            tc.tile_pool(name="ps_src", bufs=2, space="PSUM") as ps_src,
            tc.tile_pool(name="ps_sc", bufs=1, space="PSUM") as ps_sc,
        ):
            # ---- phase 0: reassemble full weight on-device ----
            if gather:
                b_in = dram.tile([WSH, 2 * H], bf16)
                w2g = dram.tile([H, 2 * H], bf16)
                nc.sync.dma_start(out=b_in, in_=d_wsh.ap())
                nc.gpsimd.collective_compute(
                    "AllGather",
                    mybir.AluOpType.bypass,
                    replica_groups=[list(range(NCORES))],
                    ins=[b_in[:].opt()],
                    outs=[w2g[:].opt()],
                )
                w2v = w2g[:]
            else:
                w2v = d_w2.ap()

            # ---- input loads: crossbar transpose-DMAs (h -> partitions) ----
            # weights: [o, g*H + hc*128 + p] -> sb_w[p, g, hc, o]
            sb_w = consts.tile([P, 2, HC, H], bf16)
            for g in range(2):
                for hc in range(HC):
                    cs = g * H + hc * P
                    nc.sync.dma_start_transpose(
                        out=sb_w[:, g, hc, :], in_=w2v[:, cs:cs + P]
                    )
            sb_htT = consts.tile([P, HC, BS, T], bf16)
            for b in range(BS):
                for hc in range(HC):
                    nc.sync.dma_start_transpose(
                        out=sb_htT[:, hc, b, :],
                        in_=d_htN.ap()[:, b, hc * P:(hc + 1) * P],
                    )
            sb_srcT = consts.tile([P, HC, BS, S], bf16)
            for b in range(BS):
                for hc in range(HC):
                    nc.sync.dma_start_transpose(
                        out=sb_srcT[:, hc, b, :],
                        in_=d_srcN.ap()[:, b, hc * P:(hc + 1) * P],
                    )
            sb_wab = consts.tile([P, OC], f32)
            nc.sync.dma_start(
                out=sb_wab, in_=d_wab.ap().rearrange("(oc p) -> p oc", p=P)
            )
            sb_va = consts.tile([P, OC, 1], f32)
            nc.sync.dma_start(
                out=sb_va,
                in_=d_va.ap().rearrange("(oc p) -> p oc", p=P).unsqueeze(2),
            )
            sb_zero = consts.tile([P, P], bf16)  # zero lhsT for psum-bank init
            nc.vector.memset(sb_zero, 0.0)

            # ---- phase 1: projections (o on partitions) ----
            ht_projb = proj.tile([P, OC, BS, T], f32)   # ht_proj + Wa_b
            src_sb = proj.tile([P, OC, BS, S], f32)     # src_proj
            for oc in range(OC):
                htp = ps_ht.tile([P, BS * T], f32, tag="htp")
                for hc in range(HC):
                    nc.tensor.matmul(
                        htp,
                        lhsT=sb_w[:, 0, hc, oc * P:(oc + 1) * P],
                        rhs=sb_htT[:, hc, :, :],
                        start=(hc == 0),
                        stop=(hc == HC - 1),
                    )
                # evacuate + fold bias (per-partition scalar add)
                nc.vector.tensor_scalar(
                    out=ht_projb[:, oc, :, :],
                    in0=htp.rearrange("p (b t) -> p b t", b=BS),
                    scalar1=sb_wab[:, oc:oc + 1],
                    scalar2=None,
                    op0=ALU.add,
                )

                srp = ps_src.tile([P, BS * S], f32, tag="srp")
                for hc in range(HC):
                    nc.tensor.matmul(
                        srp,
                        lhsT=sb_w[:, 1, hc, oc * P:(oc + 1) * P],
                        rhs=sb_srcT[:, hc, :, :],
                        start=(hc == 0),
                        stop=(hc == HC - 1),
                    )
                nc.vector.tensor_copy(
                    src_sb[:, oc, :, :], srp.rearrange("p (b s) -> p b s", b=BS)
                )

            # ---- phases 2+3: X build -> tanh -> score matmuls ----
            # scores psum: one (128, 1024) tile (2 banks) per b. Block
            # (b, k): row 32*(k%4), cols 512*(k//4)..+512. Each bank's
            # accumulation group is opened ONCE by a dummy all-zero M=128
            # matmul (start=True, writes every row -> has_written set
            # everywhere); the real M=1 Va matmuls then accumulate with
            # start=False. Correct under both whole-bank and per-partition
            # has_written-clear semantics, and keeps one group per bank.
            sc_ps = [
                ps_sc.tile([P, 1024], f32, tag=f"scb{b}", name=f"scb{b}")
                for b in range(BS)
            ]

            for b in range(BS):
                for h4 in range(2):  # open each bank's group with zeros
                    nc.tensor.matmul(
                        sc_ps[b][:, 512 * h4:512 * (h4 + 1)],
                        lhsT=sb_zero,
                        rhs=sb_srcT[:, 0:2, :, :],
                        start=True,
                        stop=False,
                        skip_group_check=True,
                    )
                for oc in range(OC):
                    ht_b = ht_projb[:, oc, b, :].unsqueeze(2).broadcast_to((P, T, S))
                    src_b = src_sb[:, oc, b, :].unsqueeze(1).broadcast_to((P, T, S))
                    x = xpool.tile([P, T, S], f32,
                                   tag="xg" if oc in GPSIMD_OCS else "xd")
                    if oc in GPSIMD_OCS:
                        nc.gpsimd.tensor_tensor(out=x, in0=ht_b, in1=src_b, op=ALU.add)
                    else:
                        nc.vector.tensor_tensor(out=x, in0=ht_b, in1=src_b, op=ALU.add)

                    h_tile = hpool.tile([P, TS], f32, tag="h")
                    nc.scalar.activation(
                        out=h_tile, in_=x.rearrange("p t s -> p (t s)"), func=AF.Tanh
                    )

                    for k in range(8):
                        j = k % 4
                        h4 = k // 4
                        nc.tensor.matmul(
                            sc_ps[b][32 * j:32 * j + 1,
                                     512 * h4:512 * (h4 + 1)],
                            lhsT=sb_va[:, oc, :],
                            rhs=h_tile[:, 512 * k:512 * (k + 1)],
                            start=False,
                            stop=(oc == OC - 1 and j == 3),
                            tile_position=(0, 32 * j),
                            skip_group_check=True,
                        )

                # ---- softmax over s for this b (cols 1024b..1024b+1024) ----
                ee = spool.tile([P, 8, S], f32, tag=f"ee{b}")
                nc.scalar.activation(
                    out=ee.rearrange("p g s -> p (g s)"),
                    in_=sc_ps[b],
                    func=AF.Exp,
                )
                sums = spool.tile([P, 8], f32, tag=f"sums{b}")
                nc.vector.reduce_sum(sums.unsqueeze(2), ee, axis=mybir.AxisListType.X)
                rec = spool.tile([P, 8], f32, tag=f"rec{b}")
                nc.vector.reciprocal(out=rec, in_=sums)
                en = spool.tile([P, 8, S], bf16, tag=f"en{b}")
                nc.vector.tensor_tensor(
                    out=en,
                    in0=ee,
                    in1=rec.unsqueeze(2).broadcast_to((P, 8, S)),
                    op=ALU.mult,
                )
                # out[t, b, s] with t = 16*k4 + 4*j + r2; en rows 32j hold
                # (k4, r2, s) at free (k4*4 + r2, s). DMA APs max 3 dims ->
                # one DMA per k4 half.
                for k4 in range(2):
                    src_view = en[0:P:32, 4 * k4:4 * (k4 + 1), :]
                    dst_view = d_out.ap().rearrange(
                        "(k4 j r2) bb s -> k4 j r2 bb s", k4=2, j=4
                    )[k4, :, :, b, :]
                    nc.sync.dma_start(out=dst_view, in_=src_view)

    nc.compile()
    return nc


def _prep_globals(h_t, src_encodings, Wa_w, Wa_b, Va_w):
    """Build the concatenated (8*shard) global input arrays directly —
    native layouts, so this is just contiguous casts + big-block permutes."""
    bf16 = ml_dtypes.bfloat16
    h_t = np.asarray(h_t, dtype=np.float32)
    src_encodings = np.asarray(src_encodings, dtype=np.float32)
    Wa_w = np.asarray(Wa_w, dtype=np.float32)
    Wa_b = np.asarray(Wa_b, dtype=np.float32)
    Va_w = np.asarray(Va_w, dtype=np.float32)

    w2sh_g = Wa_w.astype(bf16)  # (H, 2H); rows 128c..128c+128 = core c shard
    # (T, B, H) -> (NC, T, BS, H): permute of contiguous (BS*H) blocks
    htN_g = np.ascontiguousarray(
        h_t.astype(bf16).reshape(T, NCORES, BS, H).transpose(1, 0, 2, 3)
    ).reshape(NCORES * T, BS, H)
    srcN_g = np.ascontiguousarray(
        src_encodings.astype(bf16).reshape(S, NCORES, BS, H).transpose(1, 0, 2, 3)
    ).reshape(NCORES * S, BS, H)
    wab_g = np.tile(Wa_b, NCORES)
    va_g = np.tile(np.ascontiguousarray(Va_w[0]), NCORES)
    return {"w2sh": w2sh_g, "htN": htN_g, "srcN": srcN_g,
            "wab": wab_g, "va": va_g}


def _make_dispatch(nc):
    """Build a cached jit(shard_map) dispatcher mirroring
    bass_utils.run_bass_kernel_spmd's axon path (bass2jax.run_bass_via_pjrt),
    but constructed once so repeat calls hit the jit cache instead of
    retracing + relowering (~200 ms/call), and fed pre-concatenated global
    arrays so there is no per-call np.concatenate."""
    import jax
    import numpy as _np
    from jax.sharding import Mesh, PartitionSpec
    try:
        from jax import shard_map
    except ImportError:
        from jax.experimental.shard_map import shard_map
    from concourse import bass2jax, mybir

    bass2jax.install_neuronx_cc_hook()

    in_names, out_names, out_avals, zero_shapes = [], [], [], []
    for alloc in nc.m.functions[0].allocations:
        if not isinstance(alloc, mybir.MemoryLocationSet):
            continue
        name = alloc.memorylocations[0].name
        if alloc.kind == "ExternalInput":
            in_names.append(name)
        elif alloc.kind == "ExternalOutput":
            out_names.append(name)
            shape = tuple(alloc.tensor_shape)
            dtype = mybir.dt.np(alloc.dtype)
            out_avals.append(jax.core.ShapedArray(shape, dtype))
            zero_shapes.append((shape, dtype))
    n_params = len(in_names)
    all_names = tuple(in_names) + tuple(out_names)
    donate = tuple(range(n_params, n_params + len(out_names)))

    def _body(*args):
        outs = bass2jax._bass_exec_p.bind(
            *args,
            out_avals=tuple(out_avals),
            in_names=all_names,
            out_names=tuple(out_names),
            lowering_input_output_aliases=(),
            sim_require_finite=True,
            sim_require_nnan=True,
            nc=nc,
        )
        return tuple(outs)

    devices = jax.devices()[:NCORES]
    mesh = Mesh(_np.asarray(devices), ("core",))
    nin = n_params + len(out_names)
    sharded = jax.jit(
        shard_map(
            _body, mesh=mesh, in_specs=(PartitionSpec("core"),) * nin,
            out_specs=(PartitionSpec("core"),) * len(out_names), check_rep=False,
        ),
        donate_argnums=donate, keep_unused=True,
    )

    def dispatch(globals_map):
        concat_in = [globals_map[n] for n in in_names]
        concat_zeros = [
            np.zeros((NCORES * s[0], *s[1:]), dt) for s, dt in zero_shapes
        ]
        out_arrs = sharded(*concat_in, *concat_zeros)
        return {
            n: np.asarray(out_arrs[i]).reshape(NCORES, *out_avals[i].shape)
            for i, n in enumerate(out_names)
        }

    return dispatch


def _fallback_dispatch(globals_map):
    """Per-core-dict path through bass_utils.run_bass_kernel_spmd."""
    from concourse import bass_utils

    shard0 = {"w2sh": WSH, "htN": T, "srcN": S, "wab": H, "va": H}
    in_maps = [
        {n: globals_map[n][c * shard0[n]:(c + 1) * shard0[n]] for n in shard0}
        for c in range(NCORES)
    ]
    res = bass_utils.run_bass_kernel_spmd(
        _CACHE["nc"], in_maps, core_ids=list(range(NCORES))
    )
    return {"out": np.stack([r["out"] for r in res.results])}


def _gather(outs):
    # (NC, T, BS, S) bf16 -> full (T, S, B, 1) f32
    o = outs["out"].astype(np.float32)        # (NC, T, BS, S)
    full = o.transpose(1, 3, 0, 2).reshape(T, S, B)
    return np.ascontiguousarray(full[..., None])


def kernel(h_t, src_encodings, Wa_w, Wa_b, Va_w):
    if "nc" not in _CACHE:
        _CACHE["nc"] = _build_nc()
    if "dispatch" not in _CACHE:
        try:
            _CACHE["dispatch"] = _make_dispatch(_CACHE["nc"])
        except Exception:
            _CACHE["dispatch"] = _fallback_dispatch
    g = _prep_globals(h_t, src_encodings, Wa_w, Wa_b, Va_w)
    return _gather(_CACHE["dispatch"](g))


if __name__ == "__main__":
    # CoreSim check of core 0 against numpy (gather=False variant: the
    # compute pipeline is identical; the AllGather is validated on HW)
    from concourse.bass_interp import CoreSim

    rng = np.random.default_rng(0)
    w_scale = 1.0 / np.sqrt(2 * H)
    h_t = rng.standard_normal((T, B, H), dtype=np.float32)
    src = rng.standard_normal((S, B, H), dtype=np.float32)
    Wa_w = rng.standard_normal((H, 2 * H), dtype=np.float32) * w_scale
    Wa_b = rng.standard_normal((H,), dtype=np.float32) * w_scale
    Va_w = rng.standard_normal((1, H), dtype=np.float32) / np.sqrt(H)

    nc = _build_nc(gather=False)
    g = _prep_globals(h_t, src, Wa_w, Wa_b, Va_w)
    sim = CoreSim(nc)
    sim.tensor("w2")[:] = g["w2sh"]                 # full native weight
    sim.tensor("htN")[:] = g["htN"][:T]             # core 0 shard
    sim.tensor("srcN")[:] = g["srcN"][:S]
    sim.tensor("wab")[:] = g["wab"][:H]
    sim.tensor("va")[:] = g["va"][:H]
    sim.simulate(check_with_hw=False)
    got = sim.tensor("out").astype(np.float32)      # (T, BS, S)

    # numpy reference for core 0 (bf16 wire precision)
    f32 = np.float32
    bq = lambda x: x.astype(ml_dtypes.bfloat16).astype(f32)
    Wt = bq(Wa_w[:, :H]); Ws = bq(Wa_w[:, H:])
    hp = np.einsum("tbh,oh->tbo", bq(h_t[:, :BS]), Wt)
    sp = np.einsum("sbh,oh->sbo", bq(src[:, :BS]), Ws)
    hid = np.tanh(hp[:, None] + sp[None] + Wa_b)
    sc = np.einsum("tsbh,oh->tsbo", hid, Va_w)[..., 0]  # (T,S,BS)
    e = np.exp(sc - sc.max(axis=1, keepdims=True))
    ref = e / e.sum(axis=1, keepdims=True)              # (T,S,BS)
    ref_stage = ref.transpose(0, 2, 1)                  # (T,BS,S)

    err = np.abs(got - ref_stage)
    rel = err.max() / np.abs(ref_stage).max()
    print("max abs err:", err.max(), " rel:", rel)
    assert rel < 5e-3, "mismatch"
    print("SIM OK")
